# revision 32
# baseline (speedup 1.0000x reference)
"""Multi-head attention on 8 Trainium2 NeuronCores.

Problem: x[4,2048,1024] -> attention(16 heads, d=64) -> out proj -> [4,2048,1024].

Sharding (data + head/tensor parallel, per the hint): core c handles
(batch b = c//2, head half hh = c%2). Each core computes q/k/v for its 8
heads over the full 2048-row sequence, attention, and a PARTIAL output
projection y_hh = out_hh @ Wo[hh half rows] (+ bo on hh==0 via a zeros
trick). The host gathers with out[b] = y[2b] + y[2b+1] — the standard
row-parallel reduce done during unshard, so no device collective.

Per-core dataflow (bf16 matmul operands, fp32 PSUM accumulation):
  x^T fed pre-transposed from host   [c, 2048] bf16
  q^T = Wq_pair^T @ x^T              [128(2 heads), 2048]
  k^T = Wk_pair^T @ x^T              [128(2 heads), 2048]
  v   = x^T.T @ Wv (4-head waves)    [j, 4*65] with a ones column per head
  S^T = k_h^T-slices.T @ q_h^T       [j-block, i]   (K=64, row-split pair)
  expS = exp(S^T * 0.125)            ScalarE, PSUM->SBUF, [128,1024] blocks
  out^T_aug = v_aug^T @ expS^T       [65, i]  row 64 = softmax denominator Z
  out^T = out^T_aug[0:64] * (1/Z)    (Z broadcast across partitions via PE)
  y = oc^T.T @ Wo_half + bias        [i, 1024] streamed out per 512-row block

Scheduling: the attention inner loop is PSUM-drain/stream bound on the PE
(~2048 cyc/jb-block) while exp costs (1024+352)/1.2 ns on ACT, so
projections for later pairs and the output projection are pumped into the
attention loop in small generator steps to keep the PE dense (HAM warm).
The softmax normalization copies PSUM out first (poc) so the PV
accumulators recycle without stalling the in-order PE queue. out^T heads
merge into 128-partition out-proj stationaries via SBUF->SBUF DMA (h0 is
written in place by the normalize multiply); the out projection runs
per-512-row block as soon as the last pair's normalization for that block
completes, so only one block's projection remains after the last pair.
"""

import sys

if "/opt/trn_rl_repo" not in sys.path:
    sys.path.insert(0, "/opt/trn_rl_repo")

import numpy as np

B = 4
NSEQ = 2048
C = 1024          # query/model dim
HFULL = 16
H = 8             # heads per core
DH = 64
E = H * DH        # 512 inner cols per core
NI = 2048         # query rows per core
NJ = 2048         # key rows per core
NCC = C // 128    # 8 contraction chunks
NJB = NJ // 128   # 16 j blocks
NPAIR = H // 2    # 4 head pairs
NIQ = NI // 512   # 4 query tiles
SCALE = DH ** -0.5

_CACHE = {}


def _build_program():
    import concourse.bass as bass
    import concourse.mybir as mybir
    import concourse.tile as tile
    from concourse import bacc

    f32 = mybir.dt.float32
    f32r = mybir.dt.float32r
    bf16 = mybir.dt.bfloat16
    EXP = mybir.ActivationFunctionType.Exp
    MULT = mybir.AluOpType.mult
    ADD = mybir.AluOpType.add

    nc = bacc.Bacc("TRN2", target_bir_lowering=False, debug=False, num_devices=8)

    x_d = nc.dram_tensor("x", [4, NCC, 128, 512], bf16,
                         kind="ExternalInput").ap()
    wq_d = nc.dram_tensor("Wq", [C, E], bf16, kind="ExternalInput").ap()
    wk_d = nc.dram_tensor("Wk", [C, E], bf16, kind="ExternalInput").ap()
    wv_d = nc.dram_tensor("Wv", [C, E], bf16, kind="ExternalInput").ap()
    wo_d = nc.dram_tensor("Wo", [E, C], bf16, kind="ExternalInput").ap()
    bo_d = nc.dram_tensor("bo", [C], f32, kind="ExternalInput").ap()
    y_d = nc.dram_tensor("y", [NI, C], bf16, kind="ExternalOutput").ap()

    def r(ap):
        return ap.bitcast(f32r)

    with tile.TileContext(nc) as tc:
        with tc.tile_pool(name="sb", bufs=1) as sbp, \
             tc.tile_pool(name="ps", bufs=1, space="PSUM") as psp:

            # --- constants -------------------------------------------------
            ones_f32 = sbp.tile([128, 128], f32, tag="misc3", bufs=1)
            nc.gpsimd.memset(ones_f32[:], 1.0)
            onest = sbp.tile([128, 64], f32r, tag="misc2", bufs=1)
            nc.vector.tensor_copy(out=onest[:], in_=ones_f32[:, 0:64])
            ones_row = onest[64:65, :]               # [1, 64] at partition 64

            wq34 = wq_d.rearrange("(cc p) e -> p cc e", p=128)
            wk34 = wk_d.rearrange("(cc p) e -> p cc e", p=128)
            wv34 = wv_d.rearrange("(cc p) e -> p cc e", p=128)
            wo34 = wo_d.rearrange("(cc p) e -> p cc e", p=128)

            # --- phase 0: x^T load. Host packs x as [span, cc, 128, 512]
            # so each 1MB span is ONE DMA descriptor — descriptor-issue
            # serialization was dominating startup.
            xT_all = sbp.tile([128, NCC * NSEQ], bf16, tag="xT", bufs=1)
            xv = xT_all[:].rearrange("p (cc sp c) -> p cc sp c",
                                     cc=NCC, sp=4)
            for si in range(4):
                nc.sync.dma_start(
                    out=xv[:, :, si, :],
                    in_=x_d[si].rearrange("cc p c -> p cc c"))

            def xs(cc, a, b):
                return xT_all[:, cc * NSEQ + a:cc * NSEQ + b]

            vq_tiles = {}
            wvq_tiles = {}
            qkT_tiles = {}

            weight_tiles = {}

            def prefetch_pair_weights(p, eng=None):
                """Issue wq/wk DMAs for pair p well before its proj_gen
                runs, so pumped projection matmuls never head-of-line
                block the PE queue waiting on a weight transfer."""
                eng = eng or nc.gpsimd
                wqp = sbp.tile([128, C], bf16, tag="wqk", bufs=6,
                               name=f"wqp{p}")
                eng.dma_start(
                    out=wqp[:].rearrange("p (cc e) -> p cc e", cc=8),
                    in_=wq34[:, :, p * 128:(p + 1) * 128])
                wkp = sbp.tile([128, C], bf16, tag="wqk", bufs=6,
                               name=f"wkp{p}")
                eng.dma_start(
                    out=wkp[:].rearrange("p (cc e) -> p cc e", cc=8),
                    in_=wk34[:, :, p * 128:(p + 1) * 128])
                weight_tiles[p] = (wqp, wkp)

            def get_pair_tiles(p):
                wqp, wkp = weight_tiles[p]
                qT = sbp.tile([128, NI], bf16, tag="qT", bufs=2,
                              name=f"qT{p}")
                kT = sbp.tile([128, NJ], bf16, tag="kT", bufs=2,
                              name=f"kT{p}")
                qkT_tiles[p] = (qT, kT)
                return wqp, wkp, qT, kT

            def make_wave(w, eng=None):
                """Allocate wave w's Wv slice + vq tile with ones columns."""
                eng = eng or nc.gpsimd
                wvq = sbp.tile([128, 8 * 256], bf16, tag="wvq", bufs=2,
                               name=f"wvq{w}")
                wvq_tiles[w] = wvq
                eng.dma_start(
                    out=wvq[:].rearrange("p (cc e) -> p cc e", cc=8),
                    in_=wv34[:, :, w * 256:(w + 1) * 256])
                vq = sbp.tile([128, NJB * 260], bf16, tag="vq", bufs=2,
                              name=f"vq{w}")
                vq_tiles[w] = vq
                nc.vector.tensor_copy(
                    out=vq[:].rearrange("p (jb h e) -> p jb h e",
                                        jb=NJB, h=4)[:, :, :, 64:65],
                    in_=ones_f32[:, 0:64].rearrange(
                        "p (a b c) -> p a b c", a=NJB, b=4))

            def emit_qproj(p, it, wqp, qT):
                pq = psp.tile([128, 512], f32, tag="pst", bufs=2,
                              name=f"pq{p}_{it}")
                for cc in range(NCC):
                    nc.tensor.matmul(
                        pq[:], wqp[:, cc * 128:(cc + 1) * 128],
                        xs(cc, it * 512, (it + 1) * 512),
                        start=(cc == 0), stop=(cc == NCC - 1))
                    if cc in (1, 3, 5):
                        yield
                nc.vector.tensor_copy(
                    out=qT[:, it * 512:(it + 1) * 512], in_=pq[:])
                yield

            def emit_kproj(p, jt, wkp, kT):
                pk = psp.tile([128, 512], f32, tag="pst", bufs=2,
                              name=f"pk{p}_{jt}")
                for cc in range(NCC):
                    nc.tensor.matmul(
                        pk[:], wkp[:, cc * 128:(cc + 1) * 128],
                        xs(cc, jt * 512, (jt + 1) * 512),
                        start=(cc == 0), stop=(cc == NCC - 1))
                    if cc in (1, 3, 5):
                        yield
                nc.vector.tensor_copy(
                    out=kT[:, jt * 512:(jt + 1) * 512], in_=pk[:])
                yield

            def emit_vproj(w, jb):
                vq_w, wvq_w = vq_tiles[w], wvq_tiles[w]
                pv = psp.tile([128, 256], f32, tag="pst", bufs=2,
                              name=f"pv{w}_{jb}")
                for cc in range(NCC):
                    nc.tensor.matmul(
                        pv[:], xs(cc, jb * 128, (jb + 1) * 128),
                        wvq_w[:, cc * 256:(cc + 1) * 256],
                        start=(cc == 0), stop=(cc == NCC - 1))
                    if cc in (1, 3, 5):
                        yield
                nc.vector.tensor_copy(
                    out=vq_w[:].rearrange(
                        "p (jb h e) -> p jb h e", jb=NJB, h=4)
                    [:, jb, :, 0:64],
                    in_=pv[:].rearrange("p (h e) -> p h e", h=4))
                yield

            # phase 0: pair 0 q/k + wave 0 v. q/k first (their weights land
            # first on the ring), per 512-col span so the PE chases the x
            # DMA; v last (wvq lands third). wo/bias trickle in behind.
            # gpsimd ring: wq0 first (q-it0 is the first PE work), then
            # wvq0 (v fills the span-arrival slack), then wk0, bias, wo.
            prefetch_pair_weights(0)      # wq0, wk0 first on gpsimd ring
            make_wave(0)                    # wvq0 third
            wqp0, wkp0, qT0, kT0 = get_pair_tiles(0)
            prefetch_pair_weights(1, eng=nc.sync)   # behind x on sync
            make_wave(1, eng=nc.sync)
            bias = sbp.tile([128, C], f32, tag="bias", bufs=1)
            nc.gpsimd.dma_start(out=bias[:],
                                in_=bo_d[None, :].to_broadcast((128, C)))
            wo_t = sbp.tile([128, 4 * C], bf16, tag="wo", bufs=1)
            nc.gpsimd.dma_start(
                out=wo_t[:].rearrange("p (cc e) -> p cc e", cc=4),
                in_=wo34[:, :, :])
            for jt in range(4):
                for _ in emit_qproj(0, jt, wqp0, qT0):
                    pass
                for _ in emit_kproj(0, jt, wkp0, kT0):
                    pass
                for _ in emit_vproj(0, jt):
                    pass

            def v0tail_gen():
                # v-wave-0 blocks 4..15, pumped inside pair-0 iq0 so the
                # serial phase-0 shrinks by ~10us of overlap-able work.
                for jb in range(4, NJB):
                    yield from emit_vproj(0, jb)

            def proj_gen(p):
                """Pair p's q/k projections plus its slice of v-wave work,
                pumped into pair p-1's attention loop. Wave 1 (heads 4-7,
                used by pairs 2 and 3) must be complete before pair 2."""
                wqp, wkp, qT, kT = get_pair_tiles(p)
                for it in range(NIQ):
                    yield from emit_qproj(p, it, wqp, qT)
                for jt in range(NJ // 512):
                    yield from emit_kproj(p, jt, wkp, kT)
                vjbs = {1: range(0, 8), 2: range(8, 16), 3: range(0)}[p]
                for jb in vjbs:
                    yield from emit_vproj(1, jb)

            # merged attention outputs, out-proj stationaries: oc[p] rows
            # 0:64 = head 2p (written in place), 64:128 = head 2p+1 (DMA).
            oc = [sbp.tile([128, NI], bf16, tag="oc", bufs=4, name=f"oc{p}")
                  for p in range(NPAIR)]

            def norm_gen(p, iq, pocs):
                """Deferred normalization for (pair p, iq): runs as pump
                work inside the NEXT iq's attention loop so the next iq's
                first S matmul (and its exp) never queues behind it. The
                PSUM accumulators were already released by the eager poc
                copies. For pair 3, chains straight into that iq-block's
                output projection."""
                for h2 in range(2):
                    head = 2 * p + h2
                    poc = pocs[h2]
                    rf = sbp.tile([128, 512], f32, tag="rf", bufs=2,
                                  name=f"rf{head}_{iq}")
                    nc.vector.reciprocal_approx_fast(
                        out=rf[0:65, :], in_=poc[0:65, :])
                    rc = sbp.tile([128, 512], f32r, tag="rc", bufs=2,
                                  name=f"rc{head}_{iq}")
                    nc.vector.tensor_copy(out=rc[64:65, :],
                                          in_=rf[64:65, :])
                    yield
                    pz = psp.tile([128, 512], f32, tag="pst", bufs=2,
                                  name=f"pz{head}_{iq}")
                    nc.tensor.matmul(
                        pz[0:64, :], r(ones_row), r(rc[64:65, :]),
                        start=True, stop=True)
                    if h2 == 0:
                        nc.vector.tensor_tensor(
                            out=oc[p][0:64, iq * 512:(iq + 1) * 512],
                            in0=poc[0:64, :], in1=pz[0:64, :], op=MULT)
                    else:
                        ot1 = sbp.tile([64, 512], bf16, tag="ot1",
                                       bufs=2, name=f"ot1_{head}_{iq}")
                        nc.vector.tensor_tensor(
                            out=ot1[:], in0=poc[0:64, :],
                            in1=pz[0:64, :], op=MULT)
                        nc.sync.dma_start(
                            out=oc[p][64:128, iq * 512:(iq + 1) * 512],
                            in_=ot1[:])
                    yield
                if p == NPAIR - 1:
                    yield from outproj_gen(iq)

            def outproj_gen(t):
                """Output projection + bias + store for i-rows t*512..+512,
                pumped into pair 3's attention loop for iq t+1."""
                for ib2 in range(4 * t, 4 * t + 4):
                    for eh in range(2):
                        py = psp.tile([128, 512], f32, tag="pst", bufs=2,
                                      name=f"py{ib2}_{eh}")
                        for cc in range(NPAIR):
                            nc.tensor.matmul(
                                py[:],
                                oc[cc][:, ib2 * 128:(ib2 + 1) * 128],
                                wo_t[:, cc * 1024 + eh * 512:
                                     cc * 1024 + eh * 512 + 512],
                                start=(cc == 0), stop=(cc == NPAIR - 1))
                            if cc == 1:
                                yield
                        ys = sbp.tile([128, 512], bf16, tag="ys", bufs=4,
                                      name=f"ys{ib2}_{eh}")
                        nc.vector.tensor_tensor(
                            out=ys[:], in0=py[:],
                            in1=bias[:, eh * 512:(eh + 1) * 512], op=ADD)
                        nc.sync.dma_start(
                            out=y_d[ib2 * 128:(ib2 + 1) * 128,
                                    eh * 512:(eh + 1) * 512],
                            in_=ys[:])
                        yield

            gens = {}

            def pump(*keys):
                for k in keys:
                    g = gens.get(k)
                    if g is not None:
                        if next(g, "done") == "done":
                            del gens[k]
                        return

            def drain(*keys):
                for k in keys:
                    while k in gens:
                        pump(k)

            gens["V0"] = v0tail_gen()
            for p in range(NPAIR):
                qw = p // 2
                if p + 2 < NPAIR:
                    prefetch_pair_weights(p + 2)
                if p + 1 < NPAIR:
                    gens[p + 1] = proj_gen(p + 1)
                qT, kT = qkT_tiles[p]
                vq = vq_tiles[qw]

                for iq in range(NIQ):
                    po = [psp.tile([128, 512], f32, tag="pso", bufs=2,
                                   name=f"po{p}_{iq}_{h2}") for h2 in range(2)]
                    for jb in range(NJB):
                        ps = psp.tile([128, 1024], f32, tag="pss", bufs=2)
                        for h2 in range(2):
                            d0 = h2 * 64
                            nc.tensor.matmul(
                                ps[:, h2 * 512:(h2 + 1) * 512],
                                kT[d0:d0 + 64, jb * 128:(jb + 1) * 128],
                                qT[d0:d0 + 64, iq * 512:(iq + 1) * 512],
                                start=True, stop=True)
                        ex = sbp.tile([128, 1024], bf16, tag="exps", bufs=4)
                        nc.scalar.activation(ex[:], ps[:], EXP, scale=SCALE)
                        # pump sits between S and PV: PV head-of-line
                        # blocks the PE queue on exp, so fill that latency
                        # V0 has 48 yields (4 per v-block); 4 pumps/iter
                        # completes block 4+m by iter m, consumed at 4+m.
                        pump("V0")
                        pump("V0")
                        pump("V0", "N")
                        if p + 1 < NPAIR:
                            pump("V0", p + 1)
                        else:
                            pump("N")
                        for h2 in range(2):
                            hq = (p % 2) * 2 + h2
                            nc.tensor.matmul(
                                po[h2][0:65, :],
                                vq[:, jb * 260 + hq * 65:
                                   jb * 260 + hq * 65 + 65],
                                ex[:, h2 * 512:(h2 + 1) * 512],
                                start=(jb == 0), stop=(jb == NJB - 1))
                    # --- normalize: out^T = po[0:64] / po[64] ------------
                    # Eager poc copies release the PV accumulators before
                    # the next iq's PV needs the PSUM banks; the rest of
                    # the chain is deferred into the next iq's pump slots.
                    drain("V0")
                    pocs = []
                    for h2 in range(2):
                        poc = sbp.tile([128, 512], f32, tag="poc", bufs=4,
                                       name=f"poc{2 * p + h2}_{iq}")
                        nc.vector.tensor_copy(out=poc[0:65, :],
                                              in_=po[h2][0:65, :])
                        pocs.append(poc)
                    drain("N")
                    if (p, iq) == (NPAIR - 1, NIQ - 1):
                        for _ in norm_gen(p, iq, pocs):
                            pass
                    else:
                        gens["N"] = norm_gen(p, iq, pocs)
                if p + 1 < NPAIR:
                    drain(p + 1)
            drain("N")

    nc.compile()
    return nc


def _get_program():
    if "nc" not in _CACHE:
        _CACHE["nc"] = _build_program()
    return _CACHE["nc"]


def _make_in_maps(x, Wq, Wk, Wv, Wo, bo):
    import ml_dtypes
    bf = ml_dtypes.bfloat16
    x = np.asarray(x, dtype=np.float32)
    Wq = np.asarray(Wq, dtype=np.float32).astype(bf)
    Wk = np.asarray(Wk, dtype=np.float32).astype(bf)
    Wv = np.asarray(Wv, dtype=np.float32).astype(bf)
    Wo = np.asarray(Wo, dtype=np.float32).astype(bf)
    bo = np.ascontiguousarray(np.asarray(bo, dtype=np.float32))
    bo0 = np.zeros_like(bo)
    in_maps = []
    # pack x^T as [span, cc, 128, 512] so each 1MB span is one descriptor
    xTs = [np.ascontiguousarray(
        x[b].T.astype(bf).reshape(NCC, 128, 4, 512).transpose(2, 0, 1, 3))
        for b in range(B)]
    for c in range(8):
        b, hh = c // 2, c % 2
        sl = slice(hh * E, (hh + 1) * E)
        in_maps.append({
            "x": xTs[b],
            "Wq": np.ascontiguousarray(Wq[:, sl]),
            "Wk": np.ascontiguousarray(Wk[:, sl]),
            "Wv": np.ascontiguousarray(Wv[:, sl]),
            "Wo": np.ascontiguousarray(Wo[sl, :]),
            "bo": bo if hh == 0 else bo0,
        })
    return in_maps


def _assemble(results):
    out = np.empty((B, NSEQ, C), dtype=np.float32)
    for b in range(B):
        np.add(results[2 * b]["y"].astype(np.float32),
               results[2 * b + 1]["y"].astype(np.float32), out=out[b])
    return out


def kernel(x, Wq, Wk, Wv, Wo, bo):
    from concourse.bass_utils import run_bass_kernel_spmd

    nc = _get_program()
    in_maps = _make_in_maps(x, Wq, Wk, Wv, Wo, bo)
    res = run_bass_kernel_spmd(nc, in_maps, list(range(8)))
    return _assemble(res.results)


def kernel_traced(x, Wq, Wk, Wv, Wo, bo):
    """Like kernel() but also neuron-profiles; returns (out, exec_time_ns)."""
    from concourse.bass_utils import run_bass_kernel_spmd

    nc = _get_program()
    in_maps = _make_in_maps(x, Wq, Wk, Wv, Wo, bo)
    res = run_bass_kernel_spmd(nc, in_maps, list(range(8)), trace=True)
    return _assemble(res.results), res.exec_time_ns


# revision 33
# speedup vs baseline: 1.2411x; 1.2411x over previous
"""Multi-head attention on 8 Trainium2 NeuronCores.

Problem: x[4,2048,1024] -> attention(16 heads, d=64) -> out proj -> [4,2048,1024].

Sharding (data + head/tensor parallel, per the hint): core c handles
(batch b = c//2, head half hh = c%2). Each core computes q/k/v for its 8
heads over the full 2048-row sequence, attention, and a PARTIAL output
projection y_hh = out_hh @ Wo[hh half rows] (+ bo on hh==0 via a zeros
trick). The host gathers with out[b] = y[2b] + y[2b+1] — the standard
row-parallel reduce done during unshard, so no device collective.

Per-core dataflow (bf16 matmul operands, fp32 PSUM accumulation):
  x^T fed pre-transposed from host   [c, 2048] bf16
  q^T = Wq_pair^T @ x^T              [128(2 heads), 2048]
  k^T = Wk_pair^T @ x^T              [128(2 heads), 2048]
  v   = x^T.T @ Wv (4-head waves)    [j, 4*65] with a ones column per head
  S^T = k_h^T-slices.T @ q_h^T       [j-block, i]   (K=64, row-split pair)
  expS = exp(S^T * 0.125)            ScalarE, PSUM->SBUF, [128,1024] blocks
  out^T_aug = v_aug^T @ expS^T       [65, i]  row 64 = softmax denominator Z
  out^T = out^T_aug[0:64] * (1/Z)    (Z broadcast across partitions via PE)
  y = oc^T.T @ Wo_half + bias        [i, 1024] streamed out per 512-row block

Scheduling: the attention inner loop is PSUM-drain/stream bound on the PE
(~2048 cyc/jb-block) while exp costs (1024+352)/1.2 ns on ACT, so
projections for later pairs and the output projection are pumped into the
attention loop in small generator steps to keep the PE dense (HAM warm).
The softmax normalization copies PSUM out first (poc) so the PV
accumulators recycle without stalling the in-order PE queue. out^T heads
merge into 128-partition out-proj stationaries via SBUF->SBUF DMA (h0 is
written in place by the normalize multiply); the out projection runs
per-512-row block as soon as the last pair's normalization for that block
completes, so only one block's projection remains after the last pair.
"""

import sys

if "/opt/trn_rl_repo" not in sys.path:
    sys.path.insert(0, "/opt/trn_rl_repo")

import numpy as np

B = 4
NSEQ = 2048
C = 1024          # query/model dim
HFULL = 16
H = 8             # heads per core
DH = 64
E = H * DH        # 512 inner cols per core
NI = 2048         # query rows per core
NJ = 2048         # key rows per core
NCC = C // 128    # 8 contraction chunks
NJB = NJ // 128   # 16 j blocks
NPAIR = H // 2    # 4 head pairs
NIQ = NI // 512   # 4 query tiles
SCALE = DH ** -0.5

_CACHE = {}


def _build_program():
    import concourse.bass as bass
    import concourse.mybir as mybir
    import concourse.tile as tile
    from concourse import bacc

    f32 = mybir.dt.float32
    f32r = mybir.dt.float32r
    bf16 = mybir.dt.bfloat16
    EXP = mybir.ActivationFunctionType.Exp
    MULT = mybir.AluOpType.mult
    ADD = mybir.AluOpType.add

    nc = bacc.Bacc("TRN2", target_bir_lowering=False, debug=False, num_devices=8)

    x_d = nc.dram_tensor("x", [4, NCC, 128, 512], bf16,
                         kind="ExternalInput").ap()
    wq_d = nc.dram_tensor("Wq", [C, E], bf16, kind="ExternalInput").ap()
    wk_d = nc.dram_tensor("Wk", [C, E], bf16, kind="ExternalInput").ap()
    wv_d = nc.dram_tensor("Wv", [C, E], bf16, kind="ExternalInput").ap()
    wo_d = nc.dram_tensor("Wo", [E, C], bf16, kind="ExternalInput").ap()
    bo_d = nc.dram_tensor("bo", [C], f32, kind="ExternalInput").ap()
    y_d = nc.dram_tensor("y", [NI, C], bf16, kind="ExternalOutput").ap()
    zsc_d = nc.dram_tensor("z_sc", [H, NIQ, 512], f32).ap()

    def r(ap):
        return ap.bitcast(f32r)

    with tile.TileContext(nc) as tc:
        with tc.tile_pool(name="sb", bufs=1) as sbp, \
             tc.tile_pool(name="ps", bufs=1, space="PSUM") as psp:

            # --- constants -------------------------------------------------
            ones_f32 = sbp.tile([128, 128], f32, tag="misc3", bufs=1)
            nc.gpsimd.memset(ones_f32[:], 1.0)
            onest = sbp.tile([128, 64], f32r, tag="misc2", bufs=1)
            nc.vector.tensor_copy(out=onest[:], in_=ones_f32[:, 0:64])
            ones_row = onest[64:65, :]               # [1, 64] at partition 64

            wq34 = wq_d.rearrange("(cc p) e -> p cc e", p=128)
            wk34 = wk_d.rearrange("(cc p) e -> p cc e", p=128)
            wv34 = wv_d.rearrange("(cc p) e -> p cc e", p=128)
            wo34 = wo_d.rearrange("(cc p) e -> p cc e", p=128)

            # --- phase 0: x^T load. Host packs x as [span, cc, 128, 512]
            # so each 1MB span is ONE DMA descriptor — descriptor-issue
            # serialization was dominating startup.
            xT_all = sbp.tile([128, NCC * NSEQ], bf16, tag="xT", bufs=1)
            xv = xT_all[:].rearrange("p (cc sp c) -> p cc sp c",
                                     cc=NCC, sp=4)
            for si in range(4):
                nc.sync.dma_start(
                    out=xv[:, :, si, :],
                    in_=x_d[si].rearrange("cc p c -> p cc c"))

            def xs(cc, a, b):
                return xT_all[:, cc * NSEQ + a:cc * NSEQ + b]

            vq_tiles = {}
            wvq_tiles = {}
            qkT_tiles = {}

            weight_tiles = {}

            def prefetch_pair_weights(p, eng=None):
                """Issue wq/wk DMAs for pair p well before its proj_gen
                runs, so pumped projection matmuls never head-of-line
                block the PE queue waiting on a weight transfer."""
                eng = eng or nc.gpsimd
                wqp = sbp.tile([128, C], bf16, tag="wqk", bufs=6,
                               name=f"wqp{p}")
                eng.dma_start(
                    out=wqp[:].rearrange("p (cc e) -> p cc e", cc=8),
                    in_=wq34[:, :, p * 128:(p + 1) * 128])
                wkp = sbp.tile([128, C], bf16, tag="wqk", bufs=6,
                               name=f"wkp{p}")
                eng.dma_start(
                    out=wkp[:].rearrange("p (cc e) -> p cc e", cc=8),
                    in_=wk34[:, :, p * 128:(p + 1) * 128])
                weight_tiles[p] = (wqp, wkp)

            def get_pair_tiles(p):
                wqp, wkp = weight_tiles[p]
                qT = sbp.tile([128, NI], bf16, tag="qT", bufs=2,
                              name=f"qT{p}")
                kT = sbp.tile([128, NJ], bf16, tag="kT", bufs=2,
                              name=f"kT{p}")
                qkT_tiles[p] = (qT, kT)
                return wqp, wkp, qT, kT

            def make_wave(w, eng=None):
                """Allocate wave w's Wv slice + vq tile with ones columns."""
                eng = eng or nc.gpsimd
                wvq = sbp.tile([128, 8 * 256], bf16, tag="wvq", bufs=2,
                               name=f"wvq{w}")
                wvq_tiles[w] = wvq
                eng.dma_start(
                    out=wvq[:].rearrange("p (cc e) -> p cc e", cc=8),
                    in_=wv34[:, :, w * 256:(w + 1) * 256])
                vq = sbp.tile([128, NJB * 260], bf16, tag="vq", bufs=2,
                              name=f"vq{w}")
                vq_tiles[w] = vq
                nc.vector.tensor_copy(
                    out=vq[:].rearrange("p (jb h e) -> p jb h e",
                                        jb=NJB, h=4)[:, :, :, 64:65],
                    in_=ones_f32[:, 0:64].rearrange(
                        "p (a b c) -> p a b c", a=NJB, b=4))

            def emit_qproj(p, it, wqp, qT):
                pq = psp.tile([128, 512], f32, tag="pst", bufs=2,
                              name=f"pq{p}_{it}")
                for cc in range(NCC):
                    nc.tensor.matmul(
                        pq[:], wqp[:, cc * 128:(cc + 1) * 128],
                        xs(cc, it * 512, (it + 1) * 512),
                        start=(cc == 0), stop=(cc == NCC - 1))
                    if cc in (1, 3, 5):
                        yield
                nc.vector.tensor_copy(
                    out=qT[:, it * 512:(it + 1) * 512], in_=pq[:])
                yield

            def emit_kproj(p, jt, wkp, kT):
                pk = psp.tile([128, 512], f32, tag="pst", bufs=2,
                              name=f"pk{p}_{jt}")
                for cc in range(NCC):
                    nc.tensor.matmul(
                        pk[:], wkp[:, cc * 128:(cc + 1) * 128],
                        xs(cc, jt * 512, (jt + 1) * 512),
                        start=(cc == 0), stop=(cc == NCC - 1))
                    if cc in (1, 3, 5):
                        yield
                nc.vector.tensor_copy(
                    out=kT[:, jt * 512:(jt + 1) * 512], in_=pk[:])
                yield

            def emit_vproj(w, jb):
                vq_w, wvq_w = vq_tiles[w], wvq_tiles[w]
                pv = psp.tile([128, 256], f32, tag="pst", bufs=2,
                              name=f"pv{w}_{jb}")
                for cc in range(NCC):
                    nc.tensor.matmul(
                        pv[:], xs(cc, jb * 128, (jb + 1) * 128),
                        wvq_w[:, cc * 256:(cc + 1) * 256],
                        start=(cc == 0), stop=(cc == NCC - 1))
                    if cc in (1, 3, 5):
                        yield
                nc.vector.tensor_copy(
                    out=vq_w[:].rearrange(
                        "p (jb h e) -> p jb h e", jb=NJB, h=4)
                    [:, jb, :, 0:64],
                    in_=pv[:].rearrange("p (h e) -> p h e", h=4))
                yield

            # phase 0: pair 0 q/k + wave 0 v. q/k first (their weights land
            # first on the ring), per 512-col span so the PE chases the x
            # DMA; v last (wvq lands third). wo/bias trickle in behind.
            # gpsimd ring: wq0 first (q-it0 is the first PE work), then
            # wvq0 (v fills the span-arrival slack), then wk0, bias, wo.
            prefetch_pair_weights(0)      # wq0, wk0 first on gpsimd ring
            make_wave(0)                    # wvq0 third
            wqp0, wkp0, qT0, kT0 = get_pair_tiles(0)
            prefetch_pair_weights(1, eng=nc.sync)   # behind x on sync
            make_wave(1, eng=nc.sync)
            bias = sbp.tile([128, C], f32, tag="bias", bufs=1)
            nc.gpsimd.dma_start(out=bias[:],
                                in_=bo_d[None, :].to_broadcast((128, C)))
            wo_t = sbp.tile([128, 4 * C], bf16, tag="wo", bufs=1)
            nc.gpsimd.dma_start(
                out=wo_t[:].rearrange("p (cc e) -> p cc e", cc=4),
                in_=wo34[:, :, :])
            for jt in range(4):
                for _ in emit_qproj(0, jt, wqp0, qT0):
                    pass
                for _ in emit_kproj(0, jt, wkp0, kT0):
                    pass
                for _ in emit_vproj(0, jt):
                    pass

            def v0tail_gen():
                # v-wave-0 blocks 4..15, pumped inside pair-0 iq0 so the
                # serial phase-0 shrinks by ~10us of overlap-able work.
                for jb in range(4, NJB):
                    yield from emit_vproj(0, jb)

            def proj_gen(p):
                """Pair p's q/k projections plus its slice of v-wave work,
                pumped into pair p-1's attention loop. Wave 1 (heads 4-7,
                used by pairs 2 and 3) must be complete before pair 2."""
                wqp, wkp, qT, kT = get_pair_tiles(p)
                for it in range(NIQ):
                    yield from emit_qproj(p, it, wqp, qT)
                for jt in range(NJ // 512):
                    yield from emit_kproj(p, jt, wkp, kT)
                vjbs = {1: range(0, 8), 2: range(8, 16), 3: range(0)}[p]
                for jb in vjbs:
                    yield from emit_vproj(1, jb)

            # merged attention outputs, out-proj stationaries: oc[p] rows
            # 0:64 = head 2p (written in place), 64:128 = head 2p+1 (DMA).
            oc = [sbp.tile([128, NI], bf16, tag="oc", bufs=4, name=f"oc{p}")
                  for p in range(NPAIR)]

            def norm_gen(p, iq, pocs, inline=False):
                """Deferred normalization for (pair p, iq): runs as pump
                work inside the NEXT iq's attention loop so the next iq's
                first S matmul (and its exp) never queues behind it. The
                PSUM accumulators were already released by the eager poc
                copies. For pair 3, chains straight into that iq-block's
                output projection."""
                for h2 in range(2):
                    head = 2 * p + h2
                    poc = pocs[h2]
                    rf = sbp.tile([128, 512], f32, tag="rf", bufs=2,
                                  name=f"rf{head}_{iq}")
                    nc.vector.reciprocal_approx_fast(
                        out=rf[0:65, :], in_=poc[0:65, :])
                    if inline:
                        rc = sbp.tile([128, 512], f32r, tag="rc", bufs=2,
                                      name=f"rc{head}_{iq}")
                        nc.vector.tensor_copy(out=rc[64:65, :],
                                              in_=rf[64:65, :])
                        yield
                        pz = psp.tile([128, 512], f32, tag="pst", bufs=2,
                                      name=f"pz{head}_{iq}")
                        nc.tensor.matmul(
                            pz[0:64, :], r(ones_row), r(rc[64:65, :]),
                            start=True, stop=True)
                    else:
                        # bounce 1/Z through DRAM and broadcast-read it
                        # across 64 partitions: frees the PE matmul and
                        # the f32r copy; latency hides in the pump slack.
                        nc.gpsimd.dma_start(out=zsc_d[head, iq],
                                            in_=rf[64:65, :])
                        yield
                        pz = sbp.tile([128, 512], f32, tag="zbb", bufs=2,
                                      name=f"zbb{head}_{iq}")
                        nc.gpsimd.dma_start(
                            out=pz[0:64, :],
                            in_=zsc_d[head, iq][None, :].to_broadcast(
                                (64, 512)))
                    if h2 == 0:
                        nc.vector.tensor_tensor(
                            out=oc[p][0:64, iq * 512:(iq + 1) * 512],
                            in0=poc[0:64, :], in1=pz[0:64, :], op=MULT)
                    else:
                        ot1 = sbp.tile([64, 512], bf16, tag="ot1",
                                       bufs=2, name=f"ot1_{head}_{iq}")
                        nc.vector.tensor_tensor(
                            out=ot1[:], in0=poc[0:64, :],
                            in1=pz[0:64, :], op=MULT)
                        nc.sync.dma_start(
                            out=oc[p][64:128, iq * 512:(iq + 1) * 512],
                            in_=ot1[:])
                    yield
                if p == NPAIR - 1:
                    yield from outproj_gen(iq)

            def outproj_gen(t):
                """Output projection + bias + store for i-rows t*512..+512,
                pumped into pair 3's attention loop for iq t+1."""
                for ib2 in range(4 * t, 4 * t + 4):
                    for eh in range(2):
                        py = psp.tile([128, 512], f32, tag="pst", bufs=2,
                                      name=f"py{ib2}_{eh}")
                        for cc in range(NPAIR):
                            nc.tensor.matmul(
                                py[:],
                                oc[cc][:, ib2 * 128:(ib2 + 1) * 128],
                                wo_t[:, cc * 1024 + eh * 512:
                                     cc * 1024 + eh * 512 + 512],
                                start=(cc == 0), stop=(cc == NPAIR - 1))
                            if cc == 1:
                                yield
                        ys = sbp.tile([128, 512], bf16, tag="ys", bufs=4,
                                      name=f"ys{ib2}_{eh}")
                        nc.vector.tensor_tensor(
                            out=ys[:], in0=py[:],
                            in1=bias[:, eh * 512:(eh + 1) * 512], op=ADD)
                        nc.sync.dma_start(
                            out=y_d[ib2 * 128:(ib2 + 1) * 128,
                                    eh * 512:(eh + 1) * 512],
                            in_=ys[:])
                        yield

            gens = {}

            def pump(*keys):
                for k in keys:
                    g = gens.get(k)
                    if g is not None:
                        if next(g, "done") == "done":
                            del gens[k]
                        return

            def drain(*keys):
                for k in keys:
                    while k in gens:
                        pump(k)

            gens["V0"] = v0tail_gen()
            for p in range(NPAIR):
                qw = p // 2
                if p + 2 < NPAIR:
                    prefetch_pair_weights(p + 2)
                if p + 1 < NPAIR:
                    gens[p + 1] = proj_gen(p + 1)
                qT, kT = qkT_tiles[p]
                vq = vq_tiles[qw]

                for iq in range(NIQ):
                    po = [psp.tile([128, 512], f32, tag="pso", bufs=2,
                                   name=f"po{p}_{iq}_{h2}") for h2 in range(2)]
                    for jb in range(NJB):
                        ps = psp.tile([128, 1024], f32, tag="pss", bufs=2)
                        for h2 in range(2):
                            d0 = h2 * 64
                            nc.tensor.matmul(
                                ps[:, h2 * 512:(h2 + 1) * 512],
                                kT[d0:d0 + 64, jb * 128:(jb + 1) * 128],
                                qT[d0:d0 + 64, iq * 512:(iq + 1) * 512],
                                start=True, stop=True)
                        ex = sbp.tile([128, 1024], bf16, tag="exps", bufs=4)
                        nc.scalar.activation(ex[:], ps[:], EXP, scale=SCALE)
                        # pump sits between S and PV: PV head-of-line
                        # blocks the PE queue on exp, so fill that latency
                        # V0 has 48 yields (4 per v-block); 4 pumps/iter
                        # completes block 4+m by iter m, consumed at 4+m.
                        pump("V0")
                        pump("V0")
                        pump("V0", "N")
                        if p + 1 < NPAIR:
                            pump("V0", p + 1)
                        else:
                            pump("N")
                        for h2 in range(2):
                            hq = (p % 2) * 2 + h2
                            nc.tensor.matmul(
                                po[h2][0:65, :],
                                vq[:, jb * 260 + hq * 65:
                                   jb * 260 + hq * 65 + 65],
                                ex[:, h2 * 512:(h2 + 1) * 512],
                                start=(jb == 0), stop=(jb == NJB - 1))
                    # --- normalize: out^T = po[0:64] / po[64] ------------
                    # Eager poc copies release the PV accumulators before
                    # the next iq's PV needs the PSUM banks; the rest of
                    # the chain is deferred into the next iq's pump slots.
                    drain("V0")
                    pocs = []
                    for h2 in range(2):
                        poc = sbp.tile([128, 512], f32, tag="poc", bufs=4,
                                       name=f"poc{2 * p + h2}_{iq}")
                        nc.vector.tensor_copy(out=poc[0:65, :],
                                              in_=po[h2][0:65, :])
                        pocs.append(poc)
                    drain("N")
                    if (p, iq) == (NPAIR - 1, NIQ - 1):
                        for _ in norm_gen(p, iq, pocs, inline=True):
                            pass
                    else:
                        gens["N"] = norm_gen(p, iq, pocs)
                if p + 1 < NPAIR:
                    drain(p + 1)
            drain("N")

    nc.compile()
    return nc


def _get_program():
    if "nc" not in _CACHE:
        _CACHE["nc"] = _build_program()
    return _CACHE["nc"]


def _make_in_maps(x, Wq, Wk, Wv, Wo, bo):
    import ml_dtypes
    bf = ml_dtypes.bfloat16
    x = np.asarray(x, dtype=np.float32)
    Wq = np.asarray(Wq, dtype=np.float32).astype(bf)
    Wk = np.asarray(Wk, dtype=np.float32).astype(bf)
    Wv = np.asarray(Wv, dtype=np.float32).astype(bf)
    Wo = np.asarray(Wo, dtype=np.float32).astype(bf)
    bo = np.ascontiguousarray(np.asarray(bo, dtype=np.float32))
    bo0 = np.zeros_like(bo)
    in_maps = []
    # pack x^T as [span, cc, 128, 512] so each 1MB span is one descriptor
    xTs = [np.ascontiguousarray(
        x[b].T.astype(bf).reshape(NCC, 128, 4, 512).transpose(2, 0, 1, 3))
        for b in range(B)]
    for c in range(8):
        b, hh = c // 2, c % 2
        sl = slice(hh * E, (hh + 1) * E)
        in_maps.append({
            "x": xTs[b],
            "Wq": np.ascontiguousarray(Wq[:, sl]),
            "Wk": np.ascontiguousarray(Wk[:, sl]),
            "Wv": np.ascontiguousarray(Wv[:, sl]),
            "Wo": np.ascontiguousarray(Wo[sl, :]),
            "bo": bo if hh == 0 else bo0,
        })
    return in_maps


def _assemble(results):
    out = np.empty((B, NSEQ, C), dtype=np.float32)
    for b in range(B):
        np.add(results[2 * b]["y"].astype(np.float32),
               results[2 * b + 1]["y"].astype(np.float32), out=out[b])
    return out


def kernel(x, Wq, Wk, Wv, Wo, bo):
    from concourse.bass_utils import run_bass_kernel_spmd

    nc = _get_program()
    in_maps = _make_in_maps(x, Wq, Wk, Wv, Wo, bo)
    res = run_bass_kernel_spmd(nc, in_maps, list(range(8)))
    return _assemble(res.results)


def kernel_traced(x, Wq, Wk, Wv, Wo, bo):
    """Like kernel() but also neuron-profiles; returns (out, exec_time_ns)."""
    from concourse.bass_utils import run_bass_kernel_spmd

    nc = _get_program()
    in_maps = _make_in_maps(x, Wq, Wk, Wv, Wo, bo)
    res = run_bass_kernel_spmd(nc, in_maps, list(range(8)), trace=True)
    return _assemble(res.results), res.exec_time_ns
